# revision 5
# baseline (speedup 1.0000x reference)
import numpy as np

# FeaStNet mesh CVAE on 8 trn2 NeuronCores (batch-sharded; edges/params
# replicated). Scatter-free: segment_sum over sorted edge_row becomes
# per-dest-block staircase matmuls (host-built edge_w one-hots); dest-side
# gather becomes a 0/1 matmul; neighbor gathers stay real gathers but are
# split across several pmap modules because neuronxcc caps DMA ops per NEFF.
V = 5023
E = 30138
LAT = 32
BN_EPS = 1e-3
B = 64
NCORES = 8
VB = 256
NC = (V + VB - 1) // VB          # 20 dest-vertex chunks
GRP = 10                         # chunks per gather module
NG = NC // GRP                   # 2 gather calls per conv
M = 8
VP = NC * VB

_c = {}


def _host_prep(edge_row, edge_col, edge_w):
    er = np.asarray(edge_row).astype(np.int64)
    ec = np.asarray(edge_col).astype(np.int64)
    ew = np.asarray(edge_w).astype(np.float32)
    start = np.searchsorted(er, np.arange(0, (NC + 1) * VB, VB))
    maxe = int((start[1:] - start[:-1]).max())
    EB = ((maxe + 127) // 128) * 128
    colpad = np.zeros((NC, EB), np.int32)
    mask = np.zeros((NC, EB), np.float32)
    Sa = np.zeros((NC, EB, VB), np.float32)
    S1 = np.zeros((NC, EB, VB), np.float32)
    for ci in range(NC):
        s, t = int(start[ci]), int(start[ci + 1])
        n = t - s
        colpad[ci, :n] = ec[s:t]
        mask[ci, :n] = 1.0
        Sa[ci, np.arange(n), er[s:t] - ci * VB] = ew[s:t]
        S1[ci, np.arange(n), er[s:t] - ci * VB] = 1.0
    return colpad, mask, Sa, S1, EB


def kernel(x, eps, edge_row, edge_col, edge_w, params):
    import jax
    import jax.numpy as jnp
    from jax import lax

    devs = jax.devices()[:NCORES]
    assert len(devs) == NCORES

    relu = lambda t: jnp.maximum(t, 0.0)

    def bn(t, g, bt, mu, var):
        return (t - mu) * lax.rsqrt(var + BN_EPS) * g + bt

    if 'init' not in _c:
        colpad, mask, Sa, S1, EB = _host_prep(edge_row, edge_col, edge_w)
        rep = lambda a: jax.device_put_replicated(np.asarray(a), devs)
        P = {}
        for k, v in params.items():
            if isinstance(v, tuple):
                for i, t in enumerate(v):
                    P[f'{k}_{i}'] = rep(t)
            else:
                P[k] = rep(v)
        C = {}
        for g0 in range(NG):
            sl = slice(g0 * GRP, (g0 + 1) * GRP)
            C[f'cp{g0}'] = rep(colpad[sl])
            C[f'mk{g0}'] = rep(mask[sl])
            C[f'sa{g0}'] = rep(Sa[sl])
            C[f's1{g0}'] = rep(S1[sl])

        def pmap0(f, n):
            return jax.pmap(f, devices=devs, in_axes=(0,) * n)

        def f_pre(x, W0, b0, g, bt, mu, var, ue):
            h = relu(bn(x @ W0 + b0, g, bt, mu, var))
            xu = jnp.einsum('bvd,dm->bvm', h, ue)
            r = jnp.concatenate([xu, h], -1)
            return jnp.pad(r, ((0, 0), (0, VP - V), (0, 0)))

        def mk_gath(g0):
            def f(rec, cpg, mkg, sag, s1g, cvec):
                Bc = rec.shape[0]
                outs = []
                for k in range(GRP):
                    ci = g0 * GRP + k
                    gg = rec[:, cpg[k]]
                    xg, hj = gg[:, :, :M], gg[:, :, M:]
                    xuc = rec[:, ci * VB:(ci + 1) * VB, :M]
                    xr = jnp.einsum('ev,bvm->bem', s1g[k], xuc)
                    lg = xg - xr + cvec
                    q = jnp.exp(lg - jnp.max(lg, -1, keepdims=True))
                    q = q / jnp.sum(q, -1, keepdims=True)
                    q = q * mkg[k][None, :, None]
                    msg = (q[:, :, :, None] * hj[:, :, None, :]
                           ).reshape(Bc, -1, M * hj.shape[-1])
                    outs.append(jnp.einsum('ev,bex->bvx', sag[k], msg))
                return jnp.concatenate(outs, 1)
            return f

        def f_mid(p0, p1, eps, we, be, g2, bt2, mu2, v2, Wfc, bfc, Wd, bd,
                  g3, bt3, mu3, v3, ud1):
            agg = jnp.concatenate([p0, p1], 1)[:, :V]
            h = agg @ we.reshape(-1, we.shape[-1]) + be
            h = relu(bn(h, g2, bt2, mu2, v2))
            stats = jnp.mean(h, 1) @ Wfc + bfc
            mean, logvar = stats[:, :LAT], stats[:, LAT:]
            z = eps * jnp.exp(0.5 * logvar) + mean
            hd = (z @ Wd + bd).reshape(h.shape[0], V, LAT)
            hd = relu(bn(hd, g3, bt3, mu3, v3))
            xu = jnp.einsum('bvd,dm->bvm', hd, ud1)
            r = jnp.concatenate([xu, hd], -1)
            return jnp.pad(r, ((0, 0), (0, VP - V), (0, 0)))

        def f_mid2(p0, p1, wd1, bd1, g4, bt4, mu4, v4, ud2):
            agg = jnp.concatenate([p0, p1], 1)[:, :V]
            h = agg @ wd1.reshape(-1, wd1.shape[-1]) + bd1
            h = relu(bn(h, g4, bt4, mu4, v4))
            xu = jnp.einsum('bvd,dm->bvm', h, ud2)
            r = jnp.concatenate([xu, h], -1)
            return jnp.pad(r, ((0, 0), (0, VP - V), (0, 0)))

        def f_post(p0, p1, wd2, bd2, Wout, bout):
            agg = jnp.concatenate([p0, p1], 1)[:, :V]
            h = relu(agg @ wd2.reshape(-1, wd2.shape[-1]) + bd2)
            return h @ Wout + bout

        _c['init'] = dict(
            P=P, C=C,
            pre=pmap0(f_pre, 8),
            gath=[pmap0(mk_gath(g0), 6) for g0 in range(NG)],
            mid=pmap0(f_mid, 18), mid2=pmap0(f_mid2, 9),
            post=pmap0(f_post, 6))

    S = _c['init']
    P, C = S['P'], S['C']
    Bc = B // NCORES
    xs = np.ascontiguousarray(x.reshape(NCORES, Bc, V, x.shape[-1]))
    es = np.ascontiguousarray(eps.reshape(NCORES, Bc, LAT))

    def conv(rec, cvec):
        return [S['gath'][g](rec, C[f'cp{g}'], C[f'mk{g}'], C[f'sa{g}'],
                             C[f's1{g}'], cvec) for g in range(NG)]

    rec = S['pre'](jnp.asarray(xs), P['W0'], P['b0'], P['bn1_0'], P['bn1_1'],
                   P['bn1_2'], P['bn1_3'], P['ue'])
    p0, p1 = conv(rec, P['ce'])
    rec = S['mid'](p0, p1, jnp.asarray(es), P['we'], P['be'], P['bn2_0'],
                   P['bn2_1'], P['bn2_2'], P['bn2_3'], P['Wfc'], P['bfc'],
                   P['Wd'], P['bd'], P['bn3_0'], P['bn3_1'], P['bn3_2'],
                   P['bn3_3'], P['ud1'])
    p0, p1 = conv(rec, P['cd1'])
    rec = S['mid2'](p0, p1, P['wd1'], P['bd1'], P['bn4_0'], P['bn4_1'],
                    P['bn4_2'], P['bn4_3'], P['ud2'])
    p0, p1 = conv(rec, P['cd2'])
    out = S['post'](p0, p1, P['wd2'], P['bd2'], P['Wout'], P['bout'])
    return np.asarray(out).reshape(B, V, 3).astype(np.float32)
